# revision 55
# baseline (speedup 1.0000x reference)
"""Trainium2 Bass kernel for CrossStockAttention (sparse top-40 attention).

Strategy (8 NeuronCores, zero inter-core communication):
  - Data-parallel: core = (batch b, query-half). Each core owns 1024 queries of
    one batch and all of that batch's valid keys (compacted, padded to 128).
  - Host-side marshalling: queries permuted valid-first; keys compacted to
    valid-only. Permutation undone on the host after gathering.
  - Ranking trick: cosine top-k per query row is invariant to the positive
    per-row scale 1/|x_q|, so only KEYS are normalized (sim = x_q . x_k/|x_k|).
    Keys are normalized on the HOST (exact fp32), so the kernel has no norm
    phase and the sim -> topk spine starts as soon as the first DMAs land.
  - sim needs ~fp32 accuracy (f32r's ~1e-3 matmul error swaps top-40 boundary
    keys and corrupted ~2% of queries on hardware; a bf16 hi/lo split at
    ~1e-5 still swapped one boundary pair with a 7.6e-6 gap). Instead of
    4-pass fp32, sim uses a hi/lo FP16 split: q=qhi+qlo, k=khi+klo (host),
    sim ~= qhi.khi + qlo.khi + qhi.klo -- 3 passes at bf16 PE rate, error
    ~2.4e-7, and FWL-eligible weight loads.
  - Exact top-40 per valid query via 5 rounds of DVE max8 + match_replace on a
    fp32 work buffer. The work init is fused into the PSUM-drain ACT copy as
    work = selq*sim + negb, which forces invalid-query rows to -1e9 so their
    mask becomes all-ones. The mask is (work == -1e9) on DVE: removed top-40
    slots and invalid-query rows compare equal; padded keys also match but
    have V == 0 and no denominator contribution, so they are inert.
  - mask [q,k] is transposed to maskT [k,q] tile-by-tile with DMA transposes
    on the SP queue only (transposes racing copies on other queues corrupted
    results: the DMA xbar transpose/copy mode is serialized per-queue only).
  - qhi doubles as the Q-projection input; khi CANNOT double as the K/V
    projection input (those need the unnormalized keys), so xk_tb loads too.
  - Attention in transposed score layout S_T[k,q]: the softmax denominator
    arrives free as an extra all-ones column appended to V (gated by the
    valid-key indicator). exp() is batched [128,1024] per (head, key-tile);
    expm buffers rotate through 4 slots, and all mask multiplies run on DVE
    (GPSIMD's 2-input multiply is 3x slower and serialized the pipeline).
    Steady-state scores are emitted in head PAIRS so the two 64-contraction
    matmuls land on PE row-tiles (0,0)/(64,0) and overlap.
  - ctx rows are prescaled by 1/denom (DVE reciprocal_approx_fast), broadcast
    by GPSIMD; the output projection then accumulates head PAIRS (contraction
    128) plus the residual (identity matmul of x in f32r, rank-irrelevant)
    directly in PSUM. The output-projection + LayerNorm tail is pipelined
    per query-tile (drain+mean fused on DVE, variance STT, rstd =
    exp(-0.5*ln(var)) in one ACT table set, z on ACT, DMA out) so PE/DVE/ACT
    overlap instead of running four serial full passes. gamma/beta on host.
"""

import math
import numpy as np
import ml_dtypes

import concourse.bass as bass
import concourse.mybir as mybir
from concourse.tile import TileContext
from concourse import bass_utils, bacc

B, N, D, H, TOPK = 4, 2048, 512, 8, 40
DH = D // H
NQ = N // 2          # queries per core
QT = NQ // 128       # query tiles (8)
DC = D // 128        # feature chunks (4)
F32 = mybir.dt.float32
F32R = mybir.dt.float32r
F16 = mybir.dt.float16
BF16 = mybir.dt.bfloat16
AF = mybir.ActivationFunctionType
ALU = mybir.AluOpType

MASK_KT_DVE = 9      # mask-multiply key-tiles below this index on DVE (9 = all)
DEBUG_TAPS = False   # DMA intermediates to DRAM for sim-vs-hw triage


def _chunk3(x, p=128):
    """[A*p, F] -> [p, A, F] (partition-major chunking along dim0)."""
    a = x.shape[0] // p
    return np.ascontiguousarray(x.reshape(a, p, -1).transpose(1, 0, 2))


def _nchunks(total, step=512):
    out = []
    o = 0
    while o < total:
        out.append((o, min(step, total - o)))
        o += step
    return out


def build_nc(KV, VT, KVE=None, NVE=None):
    """Build the single-core program (SPMD across 8 cores, data differs).

    KVE: real (unpadded) key count -- top-k scans stop here; the padded
         tail is -1e9 by construction and can never enter the top-40.
    NVE: real valid-query bound -- mask multiplies stop here; columns
         beyond it belong to invalid queries whose mask is all-ones.
    """
    KT = KV // 128       # key tiles
    NV = VT * 128        # padded valid-query count
    KVE = KV if KVE is None else KVE
    NVE = NV if NVE is None else min(NV, (NVE + 15) // 16 * 16)
    kv_ch = _nchunks(KV)            # 512-wide chunks (projections)
    sim_ch = _nchunks(KV, 384)      # 384-wide chunks (sim: 1 PSUM bank)
    v_ch = _nchunks(H * 65)
    NPAIR = H // 2

    nc = bacc.Bacc("TRN2", target_bir_lowering=False, debug=False, num_devices=8)

    qhi_d = nc.dram_tensor("qhi", [128, DC, NQ], F16, kind="ExternalInput")
    qlo_d = nc.dram_tensor("qlo", [128, DC, NQ], F16, kind="ExternalInput")
    khi_d = nc.dram_tensor("khi", [128, DC, KV], F16, kind="ExternalInput")
    klo_d = nc.dram_tensor("klo", [128, DC, KV], F16, kind="ExternalInput")
    xk_tb_d = nc.dram_tensor("xk_tb", [128, DC, KV], BF16, kind="ExternalInput")
    wq_t_d = nc.dram_tensor("wq_t", [128, DC, D], F16, kind="ExternalInput")
    wk_t_d = nc.dram_tensor("wk_t", [128, DC, D], BF16, kind="ExternalInput")
    wv_t_d = nc.dram_tensor("wv_t", [128, DC, H * 65], BF16, kind="ExternalInput")
    wo_p_d = nc.dram_tensor("wo_p", [128, NPAIR, D], BF16, kind="ExternalInput")
    validk_b_d = nc.dram_tensor("validk_b", [1, KV], BF16, kind="ExternalInput")
    extra_rhs_d = nc.dram_tensor("extra_rhs", [1, H * 65], BF16, kind="ExternalInput")
    simbias_b_d = nc.dram_tensor("simbias_b", [1, KV], BF16, kind="ExternalInput")
    onesb_d = nc.dram_tensor("onesb", [1, 128], BF16, kind="ExternalInput")
    selq_d = nc.dram_tensor("selq", [128, VT], F32, kind="ExternalInput")
    negb_d = nc.dram_tensor("negb", [128, VT], F32, kind="ExternalInput")
    bq_col_d = nc.dram_tensor("bq_col", [128, DC], F32, kind="ExternalInput")
    bk_col_d = nc.dram_tensor("bk_col", [128, DC], F32, kind="ExternalInput")
    ident_d = nc.dram_tensor("ident", [128, 128], F16, kind="ExternalInput")
    bo_row_d = nc.dram_tensor("bo_row", [1, D], BF16, kind="ExternalInput")
    out_d = nc.dram_tensor("out", [128, QT, D], F32, kind="ExternalOutput")

    dbg = {}
    if DEBUG_TAPS:
        for nm, shape, dt in [
            ("dbg_work", [128, KV], F32), ("dbg_mask", [128, KV], BF16),
            ("dbg_maskT", [128, KT, NV], BF16),
            ("dbg_expm", [128, KT, NQ], BF16),
            ("dbg_ctx", [64, NQ], BF16), ("dbg_den", [1, NQ], F32),
            ("dbg_rrow", [1, NQ], F32), ("dbg_cpair", [128, NQ], BF16),
        ]:
            dbg[nm] = nc.dram_tensor(nm, shape, dt, kind="ExternalOutput")

    with TileContext(nc) as tc:
        with (
            tc.tile_pool(name="consts", bufs=1) as consts,
            tc.tile_pool(name="bigbuf", bufs=1) as bigbuf,
            tc.tile_pool(name="maskw", bufs=3) as maskw,
            tc.tile_pool(name="maskm", bufs=1) as maskm,
            tc.tile_pool(name="stream", bufs=2) as stream,
            tc.tile_pool(name="pairs", bufs=NPAIR) as pairs,
            tc.tile_pool(name="psim", bufs=2, space="PSUM") as psim,
            tc.tile_pool(name="pmm", bufs=2, space="PSUM") as pmm,
            tc.tile_pool(name="pcp", bufs=2, space="PSUM") as pcp,
            tc.tile_pool(name="small", bufs=1) as small,
        ):
            # ---- input loads ----
            def load(dram, shape, dtype=F32, pool=consts, tag=None):
                t = pool.tile(shape, dtype, tag=tag or dram.name)
                nc.sync.dma_start(t[:], dram.ap())
                return t

            def load_chunked(dram, shape, dtype, tag, pool=bigbuf, split=1,
                             psplit=1, cols=None, t=None):
                # split each chunk across several DMAs: per-engine DMA
                # bandwidth is ~1/16 of aggregate, so more transfers = more
                # engines streaming one tensor in parallel. psplit splits the
                # partition range (keeps per-partition lines >= 2KB, the DMA
                # efficiency knee, while doubling engine count). cols loads
                # only a column range (deferred-tail trick for qhi/qlo).
                if t is None:
                    t = pool.tile(shape, dtype, tag=tag)
                lo, hi = cols if cols is not None else (0, shape[2])
                step = (hi - lo + split - 1) // split
                pstep = 128 // psplit
                for c in range(shape[1]):
                    for o in range(lo, hi, step):
                        n = min(step, hi - o)
                        for p in range(0, 128, pstep):
                            nc.sync.dma_start(
                                t[p:p + pstep, c, o:o + n],
                                dram.ap()[p:p + pstep, c, o:o + n])
                return t

            # small consts first (they gate the sim drain); then the sim
            # operands (khi/qhi/qlo/klo feed sim -> topk, the DVE spine);
            # projection inputs and weights after.
            simbias_b = load(simbias_b_d, [1, KV], BF16)
            onesb = load(onesb_d, [1, 128], BF16)
            selq = load(selq_d, [128, VT], F32)
            negb = load(negb_d, [128, VT], F32)
            validk_b = load(validk_b_d, [1, KV], BF16)
            extra_rhs = load(extra_rhs_d, [1, H * 65], BF16)
            bq_col = load(bq_col_d, [128, DC], F32)
            bk_col = load(bk_col_d, [128, DC], F32)
            # sim-critical loads: khi/klo whole (partition-split doubles the
            # engine count at full-width lines); qhi/qlo only the valid-query
            # columns [0:NV] that sim reads -- the invalid-query tail is only
            # needed by Q-proj / the residual and loads after the weights
            khi = load_chunked(khi_d, [128, DC, KV], F16, "khi", psplit=2)
            klo = load_chunked(klo_d, [128, DC, KV], F16, "klo", psplit=2)
            qhi = load_chunked(qhi_d, [128, DC, NQ], F16, "qhi", cols=(0, NV))
            qlo = load_chunked(qlo_d, [128, DC, NQ], F16, "qlo", cols=(0, NV))
            xk_tb = load_chunked(xk_tb_d, [128, DC, KV], BF16, "xk_tb")
            wq_t = load_chunked(wq_t_d, [128, DC, D], F16, "wq_t", pool=consts,
                                split=1)
            wk_t = load_chunked(wk_t_d, [128, DC, D], BF16, "wk_t", pool=consts,
                                split=1)
            wv_t = load_chunked(wv_t_d, [128, DC, H * 65], BF16, "wv_t",
                                pool=consts, split=1)
            wo_p = load_chunked(wo_p_d, [128, NPAIR, D], BF16, "wo_p",
                                pool=consts, split=1)
            ident_b = load(ident_d, [128, 128], F16)
            bo_row = load(bo_row_d, [1, D], BF16)
            # deferred tails of qhi/qlo (invalid-query columns)
            load_chunked(qhi_d, [128, DC, NQ], F16, "qhi", cols=(NV, NQ), t=qhi)
            load_chunked(qlo_d, [128, DC, NQ], F16, "qlo", cols=(NV, NQ), t=qlo)

            eps_col = consts.tile([128, 1], F32)
            nc.vector.memset(eps_col[:], 1.0e-5)

            # ---- sim (hi/lo fp16) + exact top-40 mask + DMA-transposed maskT
            # Emission schedule interleaves the PE stream (sims / projections /
            # early scores) so the ACT exp stream starts ~15us earlier while
            # the DVE top-k spine stays fed:
            #   PE : sim0 sim1 sim2 | Qproj Kproj | sc0 sc1 sim3 sc2 sc3 sim4
            #   DVE: topk0 topk1 topk2 drainW3 topk3 drainW4 topk4 ...
            #   ACT: drains0-2, proj drains, then exps uninterrupted (the
            #        vt3/vt4 work drains go on DVE: on ACT they would queue
            #        behind ~20us of exps and stall the spine)
            scope_sim = nc.enter_named_scope("p_simtopk", False)
            maskT = bigbuf.tile([128, KT, NV], BF16, tag="maskT")
            scr8 = small.tile([128, 8], F32, tag="scr8")

            def emit_sim(vt, dve_drain=False):
                qs = slice(vt * 128, (vt + 1) * 128)
                work = maskw.tile([128, KV], F32, tag="work")
                for (o, n) in sim_ch:
                    ps = psim.tile([128, 384], F32, tag="sim")
                    nc.tensor.matmul(ps[:, :n], onesb[0:1, :],
                                     simbias_b[:, o:o + n], start=True, stop=False)
                    for c in range(DC):
                        nc.tensor.matmul(ps[:, :n], qhi[:, c, qs],
                                         khi[:, c, o:o + n],
                                         start=False, stop=False)
                    for c in range(DC):
                        nc.tensor.matmul(ps[:, :n], qlo[:, c, qs],
                                         khi[:, c, o:o + n],
                                         start=False, stop=False)
                    for c in range(DC):
                        nc.tensor.matmul(ps[:, :n], qhi[:, c, qs],
                                         klo[:, c, o:o + n],
                                         start=False, stop=(c == DC - 1))
                    # fused: work = selq*sim + negb (invalid-query rows ->
                    # -1e9, making their mask all-ones)
                    if dve_drain:
                        nc.vector.tensor_scalar(work[:, o:o + n], ps[:, :n],
                                                selq[:, vt:vt + 1],
                                                negb[:, vt:vt + 1],
                                                op0=ALU.mult, op1=ALU.add)
                    else:
                        nc.scalar.activation(work[:, o:o + n], ps[:, :n],
                                             AF.Identity,
                                             scale=selq[:, vt:vt + 1],
                                             bias=negb[:, vt:vt + 1])
                return work

            def emit_topk(vt, work):
                qs = slice(vt * 128, (vt + 1) * 128)
                for r in range(TOPK // 8):
                    nc.vector.max(scr8[:], work[:, 0:KVE])
                    nc.vector.match_replace(work[:, 0:KVE], scr8[:],
                                            work[:, 0:KVE], -1.0e9)
                if DEBUG_TAPS and vt == 0:
                    nc.sync.dma_start(dbg["dbg_work"].ap(), work[:])
                mask = maskm.tile([128, KV], BF16, tag="mask")
                nc.vector.tensor_scalar(mask[:], work[:], -1.0e9, None,
                                        op0=ALU.is_equal)
                if DEBUG_TAPS and vt == 0:
                    nc.sync.dma_start(dbg["dbg_mask"].ap(), mask[:])
                for kt in range(KT):
                    nc.sync.dma_start_transpose(
                        maskT[:, kt, qs], mask[:, kt * 128:(kt + 1) * 128])

            interleave = (VT == 5)
            if interleave:
                w0 = emit_sim(0)
                w1 = emit_sim(1)
                emit_topk(0, w0)
                w2 = emit_sim(2)
                emit_topk(1, w1)
            else:
                for vt in range(VT):
                    emit_topk(vt, emit_sim(vt))
            nc.leave_named_scope("p_simtopk", scope_sim[0], False)

            # ---- projections ----
            scope_proj = nc.enter_named_scope("p_proj", False)
            qt_sb = bigbuf.tile([128, DC, NQ], BF16, tag="qt")
            for dot in range(DC):
                ps = pmm.tile([128, NQ], F32, tag="mm")
                for (o, n) in _nchunks(NQ):
                    for c in range(DC):
                        nc.tensor.matmul(
                            ps[:, o:o + n],
                            wq_t[:, c, dot * 128:(dot + 1) * 128],
                            qhi[:, c, o:o + n],
                            start=(c == 0), stop=(c == DC - 1))
                nc.scalar.activation(qt_sb[:, dot, :], ps[:],
                                     AF.Identity, bias=bq_col[:, dot:dot + 1])

            kt_sb = bigbuf.tile([128, DC, KV], BF16, tag="kt")
            for dot in range(DC):
                for (o, n) in kv_ch:
                    ps = pmm.tile([128, NQ], F32, tag="mm")
                    for c in range(DC):
                        nc.tensor.matmul(
                            ps[:, :n],
                            wk_t[:, c, dot * 128:(dot + 1) * 128],
                            xk_tb[:, c, o:o + n],
                            start=(c == 0), stop=(c == DC - 1))
                    nc.scalar.activation(kt_sb[:, dot, o:o + n], ps[:, :n],
                                         AF.Identity, bias=bk_col[:, dot:dot + 1])
            if interleave:
                emit_topk(2, w2)
            nc.leave_named_scope("p_proj", scope_proj[0], False)

            # ---- attention ----
            scope_att = nc.enter_named_scope("p_attn", False)

            def emit_scores(h):
                hp = (h % 2) * 64
                hc = h // 2
                # 4-deep rotation; two buffers reuse dead phase-A slots
                # (khi/klo: dead after the last sim matmul; qhi/qlo must stay
                # alive -- the out-proj residual reconstructs x from them)
                tag = ["expmA", "expmB", "khi", "klo"][h % 4]
                expm = bigbuf.tile([128, KT, NQ], BF16, tag=tag)
                for kt in range(KT):
                    ks = slice(kt * 128, (kt + 1) * 128)
                    ps = pmm.tile([128, NQ], F32, tag="mm")
                    for (o, n) in _nchunks(NQ):
                        nc.tensor.matmul(
                            ps[:, o:o + n],
                            kt_sb[hp:hp + 64, hc, ks],
                            qt_sb[hp:hp + 64, hc, o:o + n],
                            start=True, stop=True)
                    nc.scalar.activation(expm[:, kt, :], ps[:], AF.Exp,
                                         scale=1.0 / math.sqrt(DH))
                return expm

            def emit_mask(h, expm):
                # first 3 key-tiles on GPSIMD (idle, runs them in parallel
                # with the DVE's 6) -- attnV's kt0 matmul chases the gpsimd
                # op, later kts chase the DVE ones
                for kt in range(KT):
                    eng = nc.gpsimd if kt < 3 else nc.vector
                    eng.tensor_mul(expm[:, kt, 0:NVE], expm[:, kt, 0:NVE],
                                   maskT[:, kt, 0:NVE])

            expm_q = [None] * 4
            expm_q[0] = emit_scores(0)
            expm_q[1] = emit_scores(1)
            if interleave:
                w3 = emit_sim(3, dve_drain=True)
                emit_topk(3, w3)
            expm_q[2] = emit_scores(2)
            expm_q[3] = emit_scores(3)
            if interleave:
                w4 = emit_sim(4, dve_drain=True)
                emit_topk(4, w4)
            if DEBUG_TAPS:
                nc.sync.dma_start(dbg["dbg_maskT"].ap(), maskT[:])

            vaug = bigbuf.tile([128, KT, H * 65], BF16, tag="vaug")
            for kt in range(KT):
                ks = slice(kt * 128, (kt + 1) * 128)
                for (o, n) in v_ch:
                    ps = pmm.tile([128, NQ], F32, tag="mm")
                    nc.tensor.matmul(ps[:, :n], validk_b[0:1, ks],
                                     extra_rhs[:, o:o + n], start=True, stop=False)
                    for c in range(DC):
                        nc.tensor.matmul(
                            ps[:, :n], xk_tb[:, c, ks],
                            wv_t[:, c, o:o + n],
                            start=False, stop=(c == DC - 1))
                    # vaug drains on DVE: ACT is saturated with exps here, and
                    # the DVE has a gap between the spine and the mask mults
                    nc.vector.tensor_copy(vaug[:, kt, o:o + n], ps[:, :n])

            def emit_av(h, expm):
                ctx64 = stream.tile([64, NQ], BF16, tag="ctx")
                rrow = stream.tile([1, NQ], F32, tag="rrow")
                for (o, n) in _nchunks(NQ):
                    cp = pcp.tile([65, 512], F32, tag="cp")
                    for kt in range(KT):
                        nc.tensor.matmul(cp[:, :n], vaug[:, kt, h * 65:(h + 1) * 65],
                                         expm[:, kt, o:o + n],
                                         start=(kt == 0), stop=(kt == KT - 1))
                    nc.vector.tensor_copy(rrow[:, o:o + n], cp[64:65, :n])
                    nc.vector.tensor_copy(ctx64[:, o:o + n], cp[0:64, :n])
                return ctx64, rrow

            def emit_fin(h, ctx64, rrow):
                # 1/denom in place on DVE (~18-bit approx)
                if DEBUG_TAPS and h == 0:
                    nc.sync.dma_start(dbg["dbg_den"].ap(), rrow[:])
                nc.vector.reciprocal_approx_fast(rrow[:], rrow[:])
                if DEBUG_TAPS and h == 0:
                    nc.sync.dma_start(dbg["dbg_rrow"].ap(), rrow[:])
                cpair = cpairs[h // 2]
                hp = (h % 2) * 64
                # broadcast on GPSIMD, multiply on DVE (a GPSIMD multiply
                # stalled the next heads' ctx copies ~7us/head via buffer WAR)
                for (o, n) in _nchunks(NQ):
                    rq64 = stream.tile([64, 512], F32, tag="rq64")
                    nc.gpsimd.partition_broadcast(rq64[:, :n], rrow[:, o:o + n],
                                                  channels=64)
                    nc.vector.tensor_mul(cpair[hp:hp + 64, o:o + n],
                                         ctx64[:, o:o + n], rq64[:, :n])
                if DEBUG_TAPS and h == 0:
                    nc.sync.dma_start(dbg["dbg_ctx"].ap(), ctx64[:])

            cpairs = []
            for j in range(NPAIR):
                cpair = pairs.tile([128, NQ], BF16, tag="cpair")
                cpairs.append(cpair)
            # prefill: out-proj bias + residual (x = qhi+qlo via transpose
            # matmuls against the fp16 identity) into att (bf16, dead xk_tb
            # slot). This fills the PE idle window between vaug and maskT
            # (which otherwise re-throttles HAM) and cuts the out-proj tail
            # from 13 to 4 matmuls per query tile. Drains go to ACT, which
            # idles here between the h0-3 and h4-7 exp streams.
            att = bigbuf.tile([128, QT, D], BF16, tag="xk_tb")
            for qt in range(QT):
                qs = slice(qt * 128, (qt + 1) * 128)
                for (o, n) in ((0, 384), (384, 128)):
                    ps = psim.tile([128, 384], F32, tag="sim")
                    nc.tensor.matmul(ps[:, :n], onesb[0:1, :],
                                     bo_row[:, o:o + n], start=True, stop=False)
                    ncc = list(range(o // 128, (o + n) // 128))
                    for c in ncc:
                        cl = c * 128 - o
                        nc.tensor.matmul(ps[:, cl:cl + 128], qhi[:, c, qs],
                                         ident_b[:], start=False, stop=False)
                        nc.tensor.matmul(ps[:, cl:cl + 128], qlo[:, c, qs],
                                         ident_b[:], start=False,
                                         stop=(c == ncc[-1]))
                    nc.scalar.copy(att[:, qt, o:o + n], ps[:, :n])

            # Post-maskT phase. PE FIFO is strict, so each steady-state
            # score pass (h+4) is emitted IMMEDIATELY after the attnV (h)
            # that frees its expm slot -- otherwise scores h4-7 queue behind
            # attnVs h0-3 that all wait on maskT, and each late head pays
            # its full exp latency unhidden (~14us/head measured).
            # fins trail by one head so the DVE never convoys behind the
            # copy-dependent reciprocal.
            for h in range(4):
                emit_mask(h, expm_q[h])
            if DEBUG_TAPS:
                nc.sync.dma_start(dbg["dbg_expm"].ap(), expm_q[0][:])
            fq = {}
            fq[0] = emit_av(0, expm_q[0])
            expm_q[0] = emit_scores(4)
            fq[1] = emit_av(1, expm_q[1])
            expm_q[1] = emit_scores(5)
            emit_fin(0, *fq[0])
            fq[2] = emit_av(2, expm_q[2])
            expm_q[2] = emit_scores(6)
            emit_fin(1, *fq[1])
            fq[3] = emit_av(3, expm_q[3])
            expm_q[3] = emit_scores(7)
            emit_fin(2, *fq[2])
            for h in range(4, H):
                emit_mask(h, expm_q[h % 4])
                fq[h] = emit_av(h, expm_q[h % 4])
                emit_fin(h - 1, *fq[h - 1])
            emit_fin(H - 1, *fq[H - 1])
            if DEBUG_TAPS:
                nc.sync.dma_start(dbg["dbg_cpair"].ap(), cpairs[0][:])
            nc.leave_named_scope("p_attn", scope_att[0], False)

            # ---- output projection + residual + LayerNorm (per-qt pipeline)
            scope_ln = nc.enter_named_scope("p_ln", False)
            musum = small.tile([128, QT], F32, tag="musum")
            muneg = small.tile([128, QT], F32, tag="muneg")
            varsum = small.tile([128, QT], F32, tag="varsum")
            rstd = small.tile([128, QT], F32, tag="rstd")
            mb = small.tile([128, QT], F32, tag="mb")
            # y = x + attended: att already holds bias+residual (prefill);
            # the out-proj here only adds the 4 head-pair matmuls and the
            # drain folds the PSUM into att in place. bf16 y adds ~4e-3
            # noise, within budget.
            def emit_outproj(qt):
                qs = slice(qt * 128, (qt + 1) * 128)
                ps = pmm.tile([128, NQ], F32, tag="mm")
                for j in range(NPAIR):
                    nc.tensor.matmul(ps[:, 0:D], cpairs[j][:, qs], wo_p[:, j, :],
                                     start=(j == 0), stop=(j == NPAIR - 1))
                nc.vector.scalar_tensor_tensor(
                    att[:, qt, :], ps[:, 0:D], 1.0, att[:, qt, :],
                    op0=ALU.mult, op1=ALU.add,
                    accum_out=musum[:, qt:qt + 1])
                nc.vector.tensor_scalar_mul(muneg[:, qt:qt + 1],
                                            musum[:, qt:qt + 1], -1.0 / D)
                vtmp = stream.tile([128, D], F32, tag="vtz")
                nc.vector.scalar_tensor_tensor(
                    vtmp[:], att[:, qt, :], muneg[:, qt:qt + 1], att[:, qt, :],
                    op0=ALU.add, op1=ALU.mult,
                    accum_out=varsum[:, qt:qt + 1])

            def emit_zbatch(qts):
                # rstd = 1/sqrt(var/D + eps): ACT Sqrt (one table set --
                # Ln/Exp alternation here thrashed) + exact DVE reciprocal.
                # Two batches of 4 so the first half's z/DMA overlap the
                # second half's out-proj matmuls.
                sl = slice(qts[0], qts[-1] + 1)
                nc.scalar.activation(rstd[:, sl], varsum[:, sl], AF.Sqrt,
                                     scale=1.0 / D, bias=eps_col[:])
                nc.vector.reciprocal(rstd[:, sl], rstd[:, sl])
                nc.vector.tensor_mul(mb[:, sl], muneg[:, sl], rstd[:, sl])
                for qt in qts:
                    # z = (y - mu) * rstd, alternating ACT/DVE so the z
                    # stream is not ACT-serial; gamma/beta on the host
                    z = stream.tile([128, D], F32, tag="vtz")
                    if qt % 2 == 0:
                        nc.scalar.activation(z[:], att[:, qt, :], AF.Identity,
                                             scale=rstd[:, qt:qt + 1],
                                             bias=mb[:, qt:qt + 1])
                    else:
                        nc.vector.tensor_scalar(z[:], att[:, qt, :],
                                                rstd[:, qt:qt + 1],
                                                mb[:, qt:qt + 1],
                                                op0=ALU.mult, op1=ALU.add)
                    nc.sync.dma_start(out_d.ap()[:, qt, :], z[:])

            # two z-batches: safe now that the prefill moved the residual
            # matmuls forward -- the exp stream ends BEFORE the out-proj
            # drains, so batch 1's DVE reciprocal no longer queues behind it
            # (that head-of-line block cost 30us in an earlier attempt)
            for qt in range(4):
                emit_outproj(qt)
            emit_zbatch([0, 1, 2, 3])
            for qt in range(4, QT):
                emit_outproj(qt)
            emit_zbatch([4, 5, 6, 7])
            nc.leave_named_scope("p_ln", scope_ln[0], False)
    nc.compile()
    return nc


def _prep_core(xb, validb, half, perm_k, KV, VT, k_maps):
    """Host-side shard prep for one core. Returns (in_map, perm_q)."""
    rows = np.arange(half * NQ, (half + 1) * NQ)
    vr = rows[validb[rows]]
    ir = rows[~validb[rows]]
    perm_q = np.concatenate([vr, ir])
    Vq = len(vr)

    xq = np.ascontiguousarray(xb[perm_q]).astype(np.float32)          # [NQ, D]
    qh = xq.astype(np.float16)
    ql = (xq - qh.astype(np.float32)).astype(np.float16)

    m = dict(k_maps)
    m["qhi"] = _chunk3(np.ascontiguousarray(qh.T))                    # [128,DC,NQ]
    m["qlo"] = _chunk3(np.ascontiguousarray(ql.T))
    iq = np.zeros((VT * 128,), np.float32)
    iq[Vq:] = 1.0
    iq = np.ascontiguousarray(iq.reshape(VT, 128).T)                  # [128, VT]
    m["selq"] = 1.0 - iq
    m["negb"] = -1.0e9 * iq
    return m, perm_q, xq


def kernel(stock_features, stock_valid_mask, in_proj_w, in_proj_b,
           out_w, out_b, ln_g, ln_b):
    x = np.asarray(stock_features, np.float32)
    valid = np.asarray(stock_valid_mask).astype(bool)
    W = np.asarray(in_proj_w, np.float32)
    bqkv = np.asarray(in_proj_b, np.float32)
    Wo = np.asarray(out_w, np.float32)
    bo = np.asarray(out_b, np.float32)
    g = np.asarray(ln_g, np.float32)
    be = np.asarray(ln_b, np.float32)

    perm_ks = [np.where(valid[b])[0] for b in range(B)]
    KV = int(math.ceil(max(len(p) for p in perm_ks) / 128.0)) * 128
    Vq_max = max(
        int(valid[b, half * NQ:(half + 1) * NQ].sum())
        for b in range(B) for half in range(2))
    VT = int(math.ceil(Vq_max / 128.0))

    Wq, Wk, Wv = W[:D], W[D:2 * D], W[2 * D:]
    bq, bk, bv = bqkv[:D], bqkv[D:2 * D], bqkv[2 * D:]
    wv_aug = np.zeros((D, H * 65), np.float32)
    rhs_aug = np.zeros((1, H * 65), np.float32)
    for h in range(H):
        wv_aug[:, h * 65:h * 65 + 64] = Wv.T[:, h * 64:(h + 1) * 64]
        rhs_aug[0, h * 65:h * 65 + 64] = bv[h * 64:(h + 1) * 64]
        rhs_aug[0, h * 65 + 64] = 1.0
    shared = {
        "wq_t": _chunk3(np.ascontiguousarray(Wq.T)).astype(np.float16),
        "wk_t": _chunk3(np.ascontiguousarray(Wk.T)).astype(ml_dtypes.bfloat16),
        "wv_t": _chunk3(wv_aug).astype(ml_dtypes.bfloat16),
        "wo_p": np.ascontiguousarray(
            Wo.T.reshape(H // 2, 128, D).transpose(1, 0, 2)
        ).astype(ml_dtypes.bfloat16),
        "extra_rhs": rhs_aug.astype(ml_dtypes.bfloat16),
        "bq_col": np.ascontiguousarray(bq.reshape(DC, 128).T),
        "bk_col": np.ascontiguousarray(bk.reshape(DC, 128).T),
        "ident": np.eye(128, dtype=np.float16),
        "bo_row": bo[None, :].astype(ml_dtypes.bfloat16),
        "onesb": np.ones((1, 128), ml_dtypes.bfloat16),
    }

    # per-batch key-side tensors (shared by the two cores of a batch)
    k_maps_b = []
    for b in range(B):
        Kv = len(perm_ks[b])
        xk = np.zeros((KV, D), np.float32)
        xk[:Kv] = x[b][perm_ks[b]]
        nrm = xk / np.maximum(
            np.linalg.norm(xk, axis=1, keepdims=True), 1e-12)
        kh = nrm.astype(np.float16)
        kl = (nrm - kh.astype(np.float32)).astype(np.float16)
        validk = np.zeros(KV, np.float32)
        validk[:Kv] = 1.0
        km = dict(shared)
        km["khi"] = _chunk3(np.ascontiguousarray(kh.T))               # [128,DC,KV]
        km["klo"] = _chunk3(np.ascontiguousarray(kl.T))
        km["xk_tb"] = _chunk3(np.ascontiguousarray(xk.T)).astype(
            ml_dtypes.bfloat16)
        km["validk_b"] = validk[None, :].astype(ml_dtypes.bfloat16)
        km["simbias_b"] = (-1.0e9 * (1.0 - validk))[None, :].astype(
            ml_dtypes.bfloat16)
        k_maps_b.append(km)

    in_maps = []
    perms = []
    for b in range(B):
        for half in range(2):
            m, perm_q, _ = _prep_core(x[b], valid[b], half, perm_ks[b],
                                      KV, VT, k_maps_b[b])
            in_maps.append(m)
            perms.append((b, perm_q))

    nc = build_nc(KV, VT, KVE=max(len(p) for p in perm_ks), NVE=Vq_max)
    res = bass_utils.run_bass_kernel_spmd(nc, in_maps, core_ids=list(range(8)))

    out = np.zeros((B, N, D), np.float32)
    for core, (b, perm_q) in enumerate(perms):
        o = np.asarray(res.results[core]["out"])      # [128, QT, D]
        out[b, perm_q] = o.transpose(1, 0, 2).reshape(NQ, D)
    return out * g[None, None, :] + be[None, None, :]


# revision 56
# speedup vs baseline: 1.2710x; 1.2710x over previous
"""Trainium2 Bass kernel for CrossStockAttention (sparse top-40 attention).

Strategy (8 NeuronCores, zero inter-core communication):
  - Data-parallel: core = (batch b, query-half). Each core owns 1024 queries of
    one batch and all of that batch's valid keys (compacted, padded to 128).
  - Host-side marshalling: queries permuted valid-first; keys compacted to
    valid-only. Permutation undone on the host after gathering.
  - Ranking trick: cosine top-k per query row is invariant to the positive
    per-row scale 1/|x_q|, so only KEYS are normalized (sim = x_q . x_k/|x_k|).
    Keys are normalized on the HOST (exact fp32), so the kernel has no norm
    phase and the sim -> topk spine starts as soon as the first DMAs land.
  - sim needs ~fp32 accuracy (f32r's ~1e-3 matmul error swaps top-40 boundary
    keys and corrupted ~2% of queries on hardware; a bf16 hi/lo split at
    ~1e-5 still swapped one boundary pair with a 7.6e-6 gap). Instead of
    4-pass fp32, sim uses a hi/lo FP16 split: q=qhi+qlo, k=khi+klo (host),
    sim ~= qhi.khi + qlo.khi + qhi.klo -- 3 passes at bf16 PE rate, error
    ~2.4e-7, and FWL-eligible weight loads.
  - Exact top-40 per valid query via 5 rounds of DVE max8 + match_replace on a
    fp32 work buffer. The work init is fused into the PSUM-drain ACT copy as
    work = selq*sim + negb, which forces invalid-query rows to -1e9 so their
    mask becomes all-ones. The mask is (work == -1e9) on DVE: removed top-40
    slots and invalid-query rows compare equal; padded keys also match but
    have V == 0 and no denominator contribution, so they are inert.
  - mask [q,k] is transposed to maskT [k,q] tile-by-tile with DMA transposes
    on the SP queue only (transposes racing copies on other queues corrupted
    results: the DMA xbar transpose/copy mode is serialized per-queue only).
  - qhi doubles as the Q-projection input; khi CANNOT double as the K/V
    projection input (those need the unnormalized keys), so xk_tb loads too.
  - Attention in transposed score layout S_T[k,q]: the softmax denominator
    arrives free as an extra all-ones column appended to V (gated by the
    valid-key indicator). exp() is batched [128,1024] per (head, key-tile);
    expm buffers rotate through 4 slots, and all mask multiplies run on DVE
    (GPSIMD's 2-input multiply is 3x slower and serialized the pipeline).
    Steady-state scores are emitted in head PAIRS so the two 64-contraction
    matmuls land on PE row-tiles (0,0)/(64,0) and overlap.
  - ctx rows are prescaled by 1/denom (DVE reciprocal_approx_fast), broadcast
    by GPSIMD; the output projection then accumulates head PAIRS (contraction
    128) plus the residual (identity matmul of x in f32r, rank-irrelevant)
    directly in PSUM. The output-projection + LayerNorm tail is pipelined
    per query-tile (drain+mean fused on DVE, variance STT, rstd =
    exp(-0.5*ln(var)) in one ACT table set, z on ACT, DMA out) so PE/DVE/ACT
    overlap instead of running four serial full passes. gamma/beta on host.
"""

import math
import numpy as np
import ml_dtypes

import concourse.bass as bass
import concourse.mybir as mybir
from concourse.tile import TileContext
from concourse import bass_utils, bacc

B, N, D, H, TOPK = 4, 2048, 512, 8, 40
DH = D // H
NQ = N // 2          # queries per core
QT = NQ // 128       # query tiles (8)
DC = D // 128        # feature chunks (4)
F32 = mybir.dt.float32
F32R = mybir.dt.float32r
F16 = mybir.dt.float16
BF16 = mybir.dt.bfloat16
AF = mybir.ActivationFunctionType
ALU = mybir.AluOpType

MASK_KT_DVE = 9      # mask-multiply key-tiles below this index on DVE (9 = all)
DEBUG_TAPS = False   # DMA intermediates to DRAM for sim-vs-hw triage


def _chunk3(x, p=128):
    """[A*p, F] -> [p, A, F] (partition-major chunking along dim0)."""
    a = x.shape[0] // p
    return np.ascontiguousarray(x.reshape(a, p, -1).transpose(1, 0, 2))


def _nchunks(total, step=512):
    out = []
    o = 0
    while o < total:
        out.append((o, min(step, total - o)))
        o += step
    return out


def build_nc(KV, VT, KVE=None, NVE=None):
    """Build the single-core program (SPMD across 8 cores, data differs).

    KVE: real (unpadded) key count -- top-k scans stop here; the padded
         tail is -1e9 by construction and can never enter the top-40.
    NVE: real valid-query bound -- mask multiplies stop here; columns
         beyond it belong to invalid queries whose mask is all-ones.
    """
    KT = KV // 128       # key tiles
    NV = VT * 128        # padded valid-query count
    KVE = KV if KVE is None else KVE
    NVE = NV if NVE is None else min(NV, (NVE + 15) // 16 * 16)
    kv_ch = _nchunks(KV)            # 512-wide chunks (projections)
    sim_ch = _nchunks(KV, 384)      # 384-wide chunks (sim: 1 PSUM bank)
    v_ch = _nchunks(H * 65)
    NPAIR = H // 2

    nc = bacc.Bacc("TRN2", target_bir_lowering=False, debug=False, num_devices=8)

    qhi_d = nc.dram_tensor("qhi", [128, DC, NQ], F16, kind="ExternalInput")
    qlo_d = nc.dram_tensor("qlo", [128, DC, NQ], F16, kind="ExternalInput")
    khi_d = nc.dram_tensor("khi", [128, DC, KV], F16, kind="ExternalInput")
    klo_d = nc.dram_tensor("klo", [128, DC, KV], F16, kind="ExternalInput")
    xk_tb_d = nc.dram_tensor("xk_tb", [128, DC, KV], BF16, kind="ExternalInput")
    wq_t_d = nc.dram_tensor("wq_t", [128, DC, D], F16, kind="ExternalInput")
    wk_t_d = nc.dram_tensor("wk_t", [128, DC, D], BF16, kind="ExternalInput")
    wv_t_d = nc.dram_tensor("wv_t", [128, DC, H * 65], BF16, kind="ExternalInput")
    wo_p_d = nc.dram_tensor("wo_p", [128, NPAIR, D], BF16, kind="ExternalInput")
    validk_b_d = nc.dram_tensor("validk_b", [1, KV], BF16, kind="ExternalInput")
    extra_rhs_d = nc.dram_tensor("extra_rhs", [1, H * 65], BF16, kind="ExternalInput")
    simbias_b_d = nc.dram_tensor("simbias_b", [1, KV], BF16, kind="ExternalInput")
    onesb_d = nc.dram_tensor("onesb", [1, 128], BF16, kind="ExternalInput")
    selq_d = nc.dram_tensor("selq", [128, VT], F32, kind="ExternalInput")
    negb_d = nc.dram_tensor("negb", [128, VT], F32, kind="ExternalInput")
    bq_col_d = nc.dram_tensor("bq_col", [128, DC], F32, kind="ExternalInput")
    bk_col_d = nc.dram_tensor("bk_col", [128, DC], F32, kind="ExternalInput")
    ident_d = nc.dram_tensor("ident", [128, 128], F16, kind="ExternalInput")
    bo_row_d = nc.dram_tensor("bo_row", [1, D], BF16, kind="ExternalInput")
    out_d = nc.dram_tensor("out", [128, QT, D], F32, kind="ExternalOutput")

    dbg = {}
    if DEBUG_TAPS:
        for nm, shape, dt in [
            ("dbg_work", [128, KV], F32), ("dbg_mask", [128, KV], BF16),
            ("dbg_maskT", [128, KT, NV], BF16),
            ("dbg_expm", [128, KT, NQ], BF16),
            ("dbg_ctx", [64, NQ], BF16), ("dbg_den", [1, NQ], F32),
            ("dbg_rrow", [1, NQ], F32), ("dbg_cpair", [128, NQ], BF16),
        ]:
            dbg[nm] = nc.dram_tensor(nm, shape, dt, kind="ExternalOutput")

    with TileContext(nc) as tc:
        with (
            tc.tile_pool(name="consts", bufs=1) as consts,
            tc.tile_pool(name="bigbuf", bufs=1) as bigbuf,
            tc.tile_pool(name="maskw", bufs=3) as maskw,
            tc.tile_pool(name="maskm", bufs=1) as maskm,
            tc.tile_pool(name="stream", bufs=2) as stream,
            tc.tile_pool(name="pairs", bufs=NPAIR) as pairs,
            tc.tile_pool(name="psim", bufs=2, space="PSUM") as psim,
            tc.tile_pool(name="pmm", bufs=2, space="PSUM") as pmm,
            tc.tile_pool(name="pcp", bufs=2, space="PSUM") as pcp,
            tc.tile_pool(name="small", bufs=1) as small,
        ):
            # ---- input loads ----
            def load(dram, shape, dtype=F32, pool=consts, tag=None):
                t = pool.tile(shape, dtype, tag=tag or dram.name)
                nc.sync.dma_start(t[:], dram.ap())
                return t

            def load_chunked(dram, shape, dtype, tag, pool=bigbuf, split=1,
                             psplit=1, cols=None, t=None):
                # split each chunk across several DMAs: per-engine DMA
                # bandwidth is ~1/16 of aggregate, so more transfers = more
                # engines streaming one tensor in parallel. psplit splits the
                # partition range (keeps per-partition lines >= 2KB, the DMA
                # efficiency knee, while doubling engine count). cols loads
                # only a column range (deferred-tail trick for qhi/qlo).
                if t is None:
                    t = pool.tile(shape, dtype, tag=tag)
                lo, hi = cols if cols is not None else (0, shape[2])
                step = (hi - lo + split - 1) // split
                pstep = 128 // psplit
                for c in range(shape[1]):
                    for o in range(lo, hi, step):
                        n = min(step, hi - o)
                        for p in range(0, 128, pstep):
                            nc.sync.dma_start(
                                t[p:p + pstep, c, o:o + n],
                                dram.ap()[p:p + pstep, c, o:o + n])
                return t

            # small consts first (they gate the sim drain); then the sim
            # operands (khi/qhi/qlo/klo feed sim -> topk, the DVE spine);
            # projection inputs and weights after.
            simbias_b = load(simbias_b_d, [1, KV], BF16)
            onesb = load(onesb_d, [1, 128], BF16)
            selq = load(selq_d, [128, VT], F32)
            negb = load(negb_d, [128, VT], F32)
            validk_b = load(validk_b_d, [1, KV], BF16)
            extra_rhs = load(extra_rhs_d, [1, H * 65], BF16)
            bq_col = load(bq_col_d, [128, DC], F32)
            bk_col = load(bk_col_d, [128, DC], F32)
            # sim-critical loads: khi/klo whole (partition-split doubles the
            # engine count at full-width lines); qhi/qlo only the valid-query
            # columns [0:NV] that sim reads -- the invalid-query tail is only
            # needed by Q-proj / the residual and loads after the weights
            khi = load_chunked(khi_d, [128, DC, KV], F16, "khi", psplit=2)
            klo = load_chunked(klo_d, [128, DC, KV], F16, "klo", psplit=2)
            qhi = load_chunked(qhi_d, [128, DC, NQ], F16, "qhi", cols=(0, NV))
            qlo = load_chunked(qlo_d, [128, DC, NQ], F16, "qlo", cols=(0, NV))
            xk_tb = load_chunked(xk_tb_d, [128, DC, KV], BF16, "xk_tb")
            wq_t = load_chunked(wq_t_d, [128, DC, D], F16, "wq_t", pool=consts,
                                split=1)
            wk_t = load_chunked(wk_t_d, [128, DC, D], BF16, "wk_t", pool=consts,
                                split=1)
            wv_t = load_chunked(wv_t_d, [128, DC, H * 65], BF16, "wv_t",
                                pool=consts, split=1)
            wo_p = load_chunked(wo_p_d, [128, NPAIR, D], BF16, "wo_p",
                                pool=consts, split=1)
            ident_b = load(ident_d, [128, 128], F16)
            bo_row = load(bo_row_d, [1, D], BF16)
            # deferred tails of qhi/qlo (invalid-query columns)
            load_chunked(qhi_d, [128, DC, NQ], F16, "qhi", cols=(NV, NQ), t=qhi)
            load_chunked(qlo_d, [128, DC, NQ], F16, "qlo", cols=(NV, NQ), t=qlo)

            eps_col = consts.tile([128, 1], F32)
            nc.vector.memset(eps_col[:], 1.0e-5)

            # ---- sim (hi/lo fp16) + exact top-40 mask + DMA-transposed maskT
            # Emission schedule interleaves the PE stream (sims / projections /
            # early scores) so the ACT exp stream starts ~15us earlier while
            # the DVE top-k spine stays fed:
            #   PE : sim0 sim1 sim2 | Qproj Kproj | sc0 sc1 sim3 sc2 sc3 sim4
            #   DVE: topk0 topk1 topk2 drainW3 topk3 drainW4 topk4 ...
            #   ACT: drains0-2, proj drains, then exps uninterrupted (the
            #        vt3/vt4 work drains go on DVE: on ACT they would queue
            #        behind ~20us of exps and stall the spine)
            scope_sim = nc.enter_named_scope("p_simtopk", False)
            maskT = bigbuf.tile([128, KT, NV], BF16, tag="maskT")
            scr8 = small.tile([128, 8], F32, tag="scr8")

            def emit_sim(vt, dve_drain=False):
                qs = slice(vt * 128, (vt + 1) * 128)
                work = maskw.tile([128, KV], F32, tag="work")
                for (o, n) in sim_ch:
                    ps = psim.tile([128, 384], F32, tag="sim")
                    nc.tensor.matmul(ps[:, :n], onesb[0:1, :],
                                     simbias_b[:, o:o + n], start=True, stop=False)
                    for c in range(DC):
                        nc.tensor.matmul(ps[:, :n], qhi[:, c, qs],
                                         khi[:, c, o:o + n],
                                         start=False, stop=False)
                    for c in range(DC):
                        nc.tensor.matmul(ps[:, :n], qlo[:, c, qs],
                                         khi[:, c, o:o + n],
                                         start=False, stop=False)
                    for c in range(DC):
                        nc.tensor.matmul(ps[:, :n], qhi[:, c, qs],
                                         klo[:, c, o:o + n],
                                         start=False, stop=(c == DC - 1))
                    # fused: work = selq*sim + negb (invalid-query rows ->
                    # -1e9, making their mask all-ones)
                    if dve_drain:
                        nc.vector.tensor_scalar(work[:, o:o + n], ps[:, :n],
                                                selq[:, vt:vt + 1],
                                                negb[:, vt:vt + 1],
                                                op0=ALU.mult, op1=ALU.add)
                    else:
                        nc.scalar.activation(work[:, o:o + n], ps[:, :n],
                                             AF.Identity,
                                             scale=selq[:, vt:vt + 1],
                                             bias=negb[:, vt:vt + 1])
                return work

            def emit_topk(vt, work):
                qs = slice(vt * 128, (vt + 1) * 128)
                for r in range(TOPK // 8):
                    nc.vector.max(scr8[:], work[:, 0:KVE])
                    nc.vector.match_replace(work[:, 0:KVE], scr8[:],
                                            work[:, 0:KVE], -1.0e9)
                if DEBUG_TAPS and vt == 0:
                    nc.sync.dma_start(dbg["dbg_work"].ap(), work[:])
                mask = maskm.tile([128, KV], BF16, tag="mask")
                nc.vector.tensor_scalar(mask[:], work[:], -1.0e9, None,
                                        op0=ALU.is_equal)
                if DEBUG_TAPS and vt == 0:
                    nc.sync.dma_start(dbg["dbg_mask"].ap(), mask[:])
                for kt in range(KT):
                    nc.sync.dma_start_transpose(
                        maskT[:, kt, qs], mask[:, kt * 128:(kt + 1) * 128])

            interleave = (VT == 5)
            if interleave:
                w0 = emit_sim(0)
                w1 = emit_sim(1)
                emit_topk(0, w0)
                w2 = emit_sim(2)
                emit_topk(1, w1)
            else:
                for vt in range(VT):
                    emit_topk(vt, emit_sim(vt))
            nc.leave_named_scope("p_simtopk", scope_sim[0], False)

            # ---- projections ----
            scope_proj = nc.enter_named_scope("p_proj", False)
            qt_sb = bigbuf.tile([128, DC, NQ], BF16, tag="qt")
            for dot in range(DC):
                ps = pmm.tile([128, NQ], F32, tag="mm")
                for (o, n) in _nchunks(NQ):
                    for c in range(DC):
                        nc.tensor.matmul(
                            ps[:, o:o + n],
                            wq_t[:, c, dot * 128:(dot + 1) * 128],
                            qhi[:, c, o:o + n],
                            start=(c == 0), stop=(c == DC - 1))
                nc.scalar.activation(qt_sb[:, dot, :], ps[:],
                                     AF.Identity, bias=bq_col[:, dot:dot + 1])

            kt_sb = bigbuf.tile([128, DC, KV], BF16, tag="kt")
            for dot in range(DC):
                for (o, n) in kv_ch:
                    ps = pmm.tile([128, NQ], F32, tag="mm")
                    for c in range(DC):
                        nc.tensor.matmul(
                            ps[:, :n],
                            wk_t[:, c, dot * 128:(dot + 1) * 128],
                            xk_tb[:, c, o:o + n],
                            start=(c == 0), stop=(c == DC - 1))
                    nc.scalar.activation(kt_sb[:, dot, o:o + n], ps[:, :n],
                                         AF.Identity, bias=bk_col[:, dot:dot + 1])
            if interleave:
                emit_topk(2, w2)
            nc.leave_named_scope("p_proj", scope_proj[0], False)

            # ---- attention ----
            scope_att = nc.enter_named_scope("p_attn", False)

            def emit_scores(h):
                hp = (h % 2) * 64
                hc = h // 2
                # 4-deep rotation; two buffers reuse dead phase-A slots
                # (khi/klo: dead after the last sim matmul; qhi/qlo must stay
                # alive -- the out-proj residual reconstructs x from them)
                tag = ["expmA", "expmB", "khi", "klo"][h % 4]
                expm = bigbuf.tile([128, KT, NQ], BF16, tag=tag)
                for kt in range(KT):
                    ks = slice(kt * 128, (kt + 1) * 128)
                    ps = pmm.tile([128, NQ], F32, tag="mm")
                    for (o, n) in _nchunks(NQ):
                        nc.tensor.matmul(
                            ps[:, o:o + n],
                            kt_sb[hp:hp + 64, hc, ks],
                            qt_sb[hp:hp + 64, hc, o:o + n],
                            start=True, stop=True)
                    nc.scalar.activation(expm[:, kt, :], ps[:], AF.Exp,
                                         scale=1.0 / math.sqrt(DH))
                return expm

            def emit_mask(h, expm):
                # all mask multiplies on DVE: ANY gpsimd op on the attnV
                # input path serializes the head pipeline (tried twice:
                # +65-77us both times)
                for kt in range(KT):
                    eng = nc.vector if kt < MASK_KT_DVE else nc.gpsimd
                    eng.tensor_mul(expm[:, kt, 0:NVE], expm[:, kt, 0:NVE],
                                   maskT[:, kt, 0:NVE])

            expm_q = [None] * 4
            expm_q[0] = emit_scores(0)
            expm_q[1] = emit_scores(1)
            if interleave:
                w3 = emit_sim(3, dve_drain=True)
                emit_topk(3, w3)
            expm_q[2] = emit_scores(2)
            expm_q[3] = emit_scores(3)
            if interleave:
                w4 = emit_sim(4, dve_drain=True)
                emit_topk(4, w4)
            if DEBUG_TAPS:
                nc.sync.dma_start(dbg["dbg_maskT"].ap(), maskT[:])

            vaug = bigbuf.tile([128, KT, H * 65], BF16, tag="vaug")
            for kt in range(KT):
                ks = slice(kt * 128, (kt + 1) * 128)
                for (o, n) in v_ch:
                    ps = pmm.tile([128, NQ], F32, tag="mm")
                    nc.tensor.matmul(ps[:, :n], validk_b[0:1, ks],
                                     extra_rhs[:, o:o + n], start=True, stop=False)
                    for c in range(DC):
                        nc.tensor.matmul(
                            ps[:, :n], xk_tb[:, c, ks],
                            wv_t[:, c, o:o + n],
                            start=False, stop=(c == DC - 1))
                    # vaug drains on DVE: ACT is saturated with exps here, and
                    # the DVE has a gap between the spine and the mask mults
                    nc.vector.tensor_copy(vaug[:, kt, o:o + n], ps[:, :n])

            def emit_av(h, expm):
                ctx64 = stream.tile([64, NQ], BF16, tag="ctx")
                rrow = stream.tile([1, NQ], F32, tag="rrow")
                for (o, n) in _nchunks(NQ):
                    cp = pcp.tile([65, 512], F32, tag="cp")
                    for kt in range(KT):
                        nc.tensor.matmul(cp[:, :n], vaug[:, kt, h * 65:(h + 1) * 65],
                                         expm[:, kt, o:o + n],
                                         start=(kt == 0), stop=(kt == KT - 1))
                    nc.vector.tensor_copy(rrow[:, o:o + n], cp[64:65, :n])
                    nc.vector.tensor_copy(ctx64[:, o:o + n], cp[0:64, :n])
                return ctx64, rrow

            def emit_fin(h, ctx64, rrow):
                # 1/denom in place on DVE (~18-bit approx)
                if DEBUG_TAPS and h == 0:
                    nc.sync.dma_start(dbg["dbg_den"].ap(), rrow[:])
                nc.vector.reciprocal_approx_fast(rrow[:], rrow[:])
                if DEBUG_TAPS and h == 0:
                    nc.sync.dma_start(dbg["dbg_rrow"].ap(), rrow[:])
                cpair = cpairs[h // 2]
                hp = (h % 2) * 64
                # broadcast on GPSIMD, multiply on DVE (a GPSIMD multiply
                # stalled the next heads' ctx copies ~7us/head via buffer WAR)
                for (o, n) in _nchunks(NQ):
                    rq64 = stream.tile([64, 512], F32, tag="rq64")
                    nc.gpsimd.partition_broadcast(rq64[:, :n], rrow[:, o:o + n],
                                                  channels=64)
                    nc.vector.tensor_mul(cpair[hp:hp + 64, o:o + n],
                                         ctx64[:, o:o + n], rq64[:, :n])
                if DEBUG_TAPS and h == 0:
                    nc.sync.dma_start(dbg["dbg_ctx"].ap(), ctx64[:])

            cpairs = []
            for j in range(NPAIR):
                cpair = pairs.tile([128, NQ], BF16, tag="cpair")
                cpairs.append(cpair)
            # prefill: out-proj bias + residual (x = qhi+qlo via transpose
            # matmuls against the fp16 identity) into att (bf16, dead xk_tb
            # slot). This fills the PE idle window between vaug and maskT
            # (which otherwise re-throttles HAM) and cuts the out-proj tail
            # from 13 to 4 matmuls per query tile. Drains go to ACT, which
            # idles here between the h0-3 and h4-7 exp streams.
            att = bigbuf.tile([128, QT, D], BF16, tag="xk_tb")
            for qt in range(QT):
                qs = slice(qt * 128, (qt + 1) * 128)
                for (o, n) in ((0, 384), (384, 128)):
                    ps = psim.tile([128, 384], F32, tag="sim")
                    nc.tensor.matmul(ps[:, :n], onesb[0:1, :],
                                     bo_row[:, o:o + n], start=True, stop=False)
                    ncc = list(range(o // 128, (o + n) // 128))
                    for c in ncc:
                        cl = c * 128 - o
                        nc.tensor.matmul(ps[:, cl:cl + 128], qhi[:, c, qs],
                                         ident_b[:], start=False, stop=False)
                        nc.tensor.matmul(ps[:, cl:cl + 128], qlo[:, c, qs],
                                         ident_b[:], start=False,
                                         stop=(c == ncc[-1]))
                    nc.scalar.copy(att[:, qt, o:o + n], ps[:, :n])

            # Post-maskT phase. PE FIFO is strict, so each steady-state
            # score pass (h+4) is emitted IMMEDIATELY after the attnV (h)
            # that frees its expm slot -- otherwise scores h4-7 queue behind
            # attnVs h0-3 that all wait on maskT, and each late head pays
            # its full exp latency unhidden (~14us/head measured).
            # fins trail by one head so the DVE never convoys behind the
            # copy-dependent reciprocal.
            for h in range(4):
                emit_mask(h, expm_q[h])
            if DEBUG_TAPS:
                nc.sync.dma_start(dbg["dbg_expm"].ap(), expm_q[0][:])
            fq = {}
            fq[0] = emit_av(0, expm_q[0])
            expm_q[0] = emit_scores(4)
            fq[1] = emit_av(1, expm_q[1])
            expm_q[1] = emit_scores(5)
            emit_fin(0, *fq[0])
            fq[2] = emit_av(2, expm_q[2])
            expm_q[2] = emit_scores(6)
            emit_fin(1, *fq[1])
            fq[3] = emit_av(3, expm_q[3])
            expm_q[3] = emit_scores(7)
            emit_fin(2, *fq[2])
            for h in range(4, H):
                emit_mask(h, expm_q[h % 4])
                fq[h] = emit_av(h, expm_q[h % 4])
                emit_fin(h - 1, *fq[h - 1])
            emit_fin(H - 1, *fq[H - 1])
            if DEBUG_TAPS:
                nc.sync.dma_start(dbg["dbg_cpair"].ap(), cpairs[0][:])
            nc.leave_named_scope("p_attn", scope_att[0], False)

            # ---- output projection + residual + LayerNorm (per-qt pipeline)
            scope_ln = nc.enter_named_scope("p_ln", False)
            musum = small.tile([128, QT], F32, tag="musum")
            muneg = small.tile([128, QT], F32, tag="muneg")
            varsum = small.tile([128, QT], F32, tag="varsum")
            rstd = small.tile([128, QT], F32, tag="rstd")
            mb = small.tile([128, QT], F32, tag="mb")
            # y = x + attended: att already holds bias+residual (prefill);
            # the out-proj here only adds the 4 head-pair matmuls and the
            # drain folds the PSUM into att in place. bf16 y adds ~4e-3
            # noise, within budget.
            def emit_outproj(qt):
                qs = slice(qt * 128, (qt + 1) * 128)
                ps = pmm.tile([128, NQ], F32, tag="mm")
                for j in range(NPAIR):
                    nc.tensor.matmul(ps[:, 0:D], cpairs[j][:, qs], wo_p[:, j, :],
                                     start=(j == 0), stop=(j == NPAIR - 1))
                nc.vector.scalar_tensor_tensor(
                    att[:, qt, :], ps[:, 0:D], 1.0, att[:, qt, :],
                    op0=ALU.mult, op1=ALU.add,
                    accum_out=musum[:, qt:qt + 1])
                nc.vector.tensor_scalar_mul(muneg[:, qt:qt + 1],
                                            musum[:, qt:qt + 1], -1.0 / D)
                vtmp = stream.tile([128, D], F32, tag="vtz")
                nc.vector.scalar_tensor_tensor(
                    vtmp[:], att[:, qt, :], muneg[:, qt:qt + 1], att[:, qt, :],
                    op0=ALU.add, op1=ALU.mult,
                    accum_out=varsum[:, qt:qt + 1])

            def emit_zbatch(qts):
                # rstd = 1/sqrt(var/D + eps): ACT Sqrt (one table set --
                # Ln/Exp alternation here thrashed) + exact DVE reciprocal.
                # Two batches of 4 so the first half's z/DMA overlap the
                # second half's out-proj matmuls.
                sl = slice(qts[0], qts[-1] + 1)
                nc.scalar.activation(rstd[:, sl], varsum[:, sl], AF.Sqrt,
                                     scale=1.0 / D, bias=eps_col[:])
                nc.vector.reciprocal(rstd[:, sl], rstd[:, sl])
                nc.vector.tensor_mul(mb[:, sl], muneg[:, sl], rstd[:, sl])
                for qt in qts:
                    # z = (y - mu) * rstd, alternating ACT/DVE so the z
                    # stream is not ACT-serial; gamma/beta on the host
                    z = stream.tile([128, D], F32, tag="vtz")
                    if qt % 2 == 0:
                        nc.scalar.activation(z[:], att[:, qt, :], AF.Identity,
                                             scale=rstd[:, qt:qt + 1],
                                             bias=mb[:, qt:qt + 1])
                    else:
                        nc.vector.tensor_scalar(z[:], att[:, qt, :],
                                                rstd[:, qt:qt + 1],
                                                mb[:, qt:qt + 1],
                                                op0=ALU.mult, op1=ALU.add)
                    nc.sync.dma_start(out_d.ap()[:, qt, :], z[:])

            # two z-batches: safe now that the prefill moved the residual
            # matmuls forward -- the exp stream ends BEFORE the out-proj
            # drains, so batch 1's DVE reciprocal no longer queues behind it
            # (that head-of-line block cost 30us in an earlier attempt)
            for qt in range(4):
                emit_outproj(qt)
            emit_zbatch([0, 1, 2, 3])
            for qt in range(4, QT):
                emit_outproj(qt)
            emit_zbatch([4, 5, 6, 7])
            nc.leave_named_scope("p_ln", scope_ln[0], False)
    nc.compile()
    return nc


def _prep_core(xb, validb, half, perm_k, KV, VT, k_maps):
    """Host-side shard prep for one core. Returns (in_map, perm_q)."""
    rows = np.arange(half * NQ, (half + 1) * NQ)
    vr = rows[validb[rows]]
    ir = rows[~validb[rows]]
    perm_q = np.concatenate([vr, ir])
    Vq = len(vr)

    xq = np.ascontiguousarray(xb[perm_q]).astype(np.float32)          # [NQ, D]
    qh = xq.astype(np.float16)
    ql = (xq - qh.astype(np.float32)).astype(np.float16)

    m = dict(k_maps)
    m["qhi"] = _chunk3(np.ascontiguousarray(qh.T))                    # [128,DC,NQ]
    m["qlo"] = _chunk3(np.ascontiguousarray(ql.T))
    iq = np.zeros((VT * 128,), np.float32)
    iq[Vq:] = 1.0
    iq = np.ascontiguousarray(iq.reshape(VT, 128).T)                  # [128, VT]
    m["selq"] = 1.0 - iq
    m["negb"] = -1.0e9 * iq
    return m, perm_q, xq


def kernel(stock_features, stock_valid_mask, in_proj_w, in_proj_b,
           out_w, out_b, ln_g, ln_b):
    x = np.asarray(stock_features, np.float32)
    valid = np.asarray(stock_valid_mask).astype(bool)
    W = np.asarray(in_proj_w, np.float32)
    bqkv = np.asarray(in_proj_b, np.float32)
    Wo = np.asarray(out_w, np.float32)
    bo = np.asarray(out_b, np.float32)
    g = np.asarray(ln_g, np.float32)
    be = np.asarray(ln_b, np.float32)

    perm_ks = [np.where(valid[b])[0] for b in range(B)]
    KV = int(math.ceil(max(len(p) for p in perm_ks) / 128.0)) * 128
    Vq_max = max(
        int(valid[b, half * NQ:(half + 1) * NQ].sum())
        for b in range(B) for half in range(2))
    VT = int(math.ceil(Vq_max / 128.0))

    Wq, Wk, Wv = W[:D], W[D:2 * D], W[2 * D:]
    bq, bk, bv = bqkv[:D], bqkv[D:2 * D], bqkv[2 * D:]
    wv_aug = np.zeros((D, H * 65), np.float32)
    rhs_aug = np.zeros((1, H * 65), np.float32)
    for h in range(H):
        wv_aug[:, h * 65:h * 65 + 64] = Wv.T[:, h * 64:(h + 1) * 64]
        rhs_aug[0, h * 65:h * 65 + 64] = bv[h * 64:(h + 1) * 64]
        rhs_aug[0, h * 65 + 64] = 1.0
    shared = {
        "wq_t": _chunk3(np.ascontiguousarray(Wq.T)).astype(np.float16),
        "wk_t": _chunk3(np.ascontiguousarray(Wk.T)).astype(ml_dtypes.bfloat16),
        "wv_t": _chunk3(wv_aug).astype(ml_dtypes.bfloat16),
        "wo_p": np.ascontiguousarray(
            Wo.T.reshape(H // 2, 128, D).transpose(1, 0, 2)
        ).astype(ml_dtypes.bfloat16),
        "extra_rhs": rhs_aug.astype(ml_dtypes.bfloat16),
        "bq_col": np.ascontiguousarray(bq.reshape(DC, 128).T),
        "bk_col": np.ascontiguousarray(bk.reshape(DC, 128).T),
        "ident": np.eye(128, dtype=np.float16),
        "bo_row": bo[None, :].astype(ml_dtypes.bfloat16),
        "onesb": np.ones((1, 128), ml_dtypes.bfloat16),
    }

    # per-batch key-side tensors (shared by the two cores of a batch)
    k_maps_b = []
    for b in range(B):
        Kv = len(perm_ks[b])
        xk = np.zeros((KV, D), np.float32)
        xk[:Kv] = x[b][perm_ks[b]]
        nrm = xk / np.maximum(
            np.linalg.norm(xk, axis=1, keepdims=True), 1e-12)
        kh = nrm.astype(np.float16)
        kl = (nrm - kh.astype(np.float32)).astype(np.float16)
        validk = np.zeros(KV, np.float32)
        validk[:Kv] = 1.0
        km = dict(shared)
        km["khi"] = _chunk3(np.ascontiguousarray(kh.T))               # [128,DC,KV]
        km["klo"] = _chunk3(np.ascontiguousarray(kl.T))
        km["xk_tb"] = _chunk3(np.ascontiguousarray(xk.T)).astype(
            ml_dtypes.bfloat16)
        km["validk_b"] = validk[None, :].astype(ml_dtypes.bfloat16)
        km["simbias_b"] = (-1.0e9 * (1.0 - validk))[None, :].astype(
            ml_dtypes.bfloat16)
        k_maps_b.append(km)

    in_maps = []
    perms = []
    for b in range(B):
        for half in range(2):
            m, perm_q, _ = _prep_core(x[b], valid[b], half, perm_ks[b],
                                      KV, VT, k_maps_b[b])
            in_maps.append(m)
            perms.append((b, perm_q))

    nc = build_nc(KV, VT, KVE=max(len(p) for p in perm_ks), NVE=Vq_max)
    res = bass_utils.run_bass_kernel_spmd(nc, in_maps, core_ids=list(range(8)))

    out = np.zeros((B, N, D), np.float32)
    for core, (b, perm_q) in enumerate(perms):
        o = np.asarray(res.results[core]["out"])      # [128, QT, D]
        out[b, perm_q] = o.transpose(1, 0, 2).reshape(NQ, D)
    return out * g[None, None, :] + be[None, None, :]
